# revision 8
# baseline (speedup 1.0000x reference)
"""CrossGraphConvolution kernel for Trainium2 (Bass/Tile), 8-core SPMD.

Problem: B=128 graph pairs, NPG=32 nodes per side per graph, D=OUT=128.
Edges are dense block-bipartite within each graph pair (left i <-> right j).

Math per graph pair (both directions share the cosine matrix):
  C[i,j]   = relu(cos(xl_i, xr_j))            (32x32 per graph)
  nc[i,j]  = C/(rowsum(C)+32*eps)  -> gl_i = sum_j nc*xr_j
  nc'[i,j] = C/(colsum(C)+32*eps)  -> gr_j = sum_i nc'*xl_i
  out1[i,o] = (sum_d xl*gl*w2[o]) / sqrt((sum_d xl^2*w2[o])+e) /
              sqrt((sum_d gl^2*w2[o])+e),  w2 = weight**2; same for out2.

Sharding: data-parallel over graphs; core k handles graphs [16k, 16k+16),
processed as 4 "blocks" of 4 graphs = 128 nodes per side, batched into
[128, 512] tiles wherever the free dim allows.

Normalization trick: keep C completely unscaled (C0 = relu(S_raw)*mask);
fold 1/|x| scales into per-partition row scalings of C0 / C0^T, and the
row-sum denominators via an appended ones-reduction matmul; fold the final
1/D into g via a gpsimd partition-broadcast multiply, so every later
operand is the exact ("raw") quantity and no per-free-element broadcast
scaling is ever needed.
"""

import sys

import numpy as np

for _p in ("/opt/trn_rl_repo",):
    if _p not in sys.path:
        sys.path.insert(0, _p)

B = 128
NPG = 32
D = 128
OUT = 128
EPS = 1e-6
NCORES = 8
GPC = B // NCORES          # graphs per core = 16
NPC = GPC * NPG            # nodes per side per core = 512
BLK = 128                  # nodes per block (4 graphs)
NBLK = NPC // BLK          # blocks per core = 4

_CACHE = {}


def _build_bass():
    import concourse.bass as bass
    import concourse.bacc as bacc
    import concourse.tile as tile
    from concourse import mybir
    from concourse.bass import ts
    from concourse.masks import make_identity

    f32 = mybir.dt.float32
    Sqrt = mybir.ActivationFunctionType.Sqrt
    Relu = mybir.ActivationFunctionType.Relu

    nc = bacc.Bacc(None)
    xl_d = nc.dram_tensor("xl", [NPC, D], f32, kind="ExternalInput")
    xr_d = nc.dram_tensor("xr", [NPC, D], f32, kind="ExternalInput")
    w2t_d = nc.dram_tensor("w2t", [D, OUT], f32, kind="ExternalInput")
    mask_d = nc.dram_tensor("mask4", [BLK, NPC], f32, kind="ExternalInput")
    out1_d = nc.dram_tensor("out1", [NPC, OUT], f32, kind="ExternalOutput")
    out2_d = nc.dram_tensor("out2", [NPC, OUT], f32, kind="ExternalOutput")

    with tile.TileContext(nc) as tc:
        with (
            tc.tile_pool(name="const", bufs=1) as const,
            tc.tile_pool(name="sb", bufs=1) as sb,
            tc.tile_pool(name="tp", bufs=2, space="PSUM") as tp,
            tc.tile_pool(name="big", bufs=2, space="PSUM") as big,
            tc.tile_pool(name="ein", bufs=3, space="PSUM") as ein,
            tc.tile_pool(name="pss", bufs=1, space="PSUM") as pss,
        ):
            ident = const.tile([128, 128], f32, tag="ident")
            make_identity(nc, ident)
            w2t = const.tile([D, OUT], f32, tag="w2t")
            nc.gpsimd.dma_start(out=w2t, in_=w2t_d[:])
            mask = const.tile([BLK, NPC], f32, tag="mask")
            nc.gpsimd.dma_start(out=mask, in_=mask_d[:])
            ones_col = const.tile([128, 1], f32, tag="ones")
            nc.vector.memset(ones_col, 1.0)
            eps_col = const.tile([128, 1], f32, tag="eps")
            nc.vector.memset(eps_col, EPS)
            # warm-up transposes: absorb the Pool (identity-gen) and the
            # w2t DMA-queue waits on PE so no later matmul needs >1 wait
            scrap_ps = tp.tile([128, 128], f32, tag="tp")
            nc.tensor.transpose(out=scrap_ps, in_=ident, identity=ident)
            scrap2_ps = tp.tile([128, 128], f32, tag="tp")
            nc.tensor.transpose(out=scrap2_ps, in_=w2t, identity=ident)

            def load_side(xdram, name):
                # natural layout [node-in-block, block, d], one DMA
                x_nat = sb.tile([BLK, NBLK, D], f32, tag=f"xnat_{name}")
                nc.gpsimd.dma_start(
                    out=x_nat, in_=xdram[:].rearrange("(b p) d -> p b d", p=BLK)
                )
                # transposed layout [d, node] for all blocks
                xT_ps = tp.tile([128, NPC], f32, tag="tp")
                for b in range(NBLK):
                    nc.tensor.transpose(
                        out=xT_ps[:, ts(b, BLK)], in_=x_nat[:, b, :], identity=ident
                    )
                xT = sb.tile([128, NPC], f32, tag=f"xT_{name}")
                nc.scalar.copy(out=xT, in_=xT_ps)
                x2T = sb.tile([128, NPC], f32, tag=f"x2T_{name}")
                nc.vector.tensor_mul(x2T, xT, xT)
                # squared norms as a [1, 512] row (for the +32eps*|x| term)
                nsq_row = pss.tile([1, NPC], f32, tag="small")
                nc.tensor.matmul(
                    nsq_row, lhsT=ones_col, rhs=x2T, start=True, stop=True
                )
                neps = sb.tile([1, NPC], f32, tag=f"neps_{name}")
                nc.scalar.activation(
                    neps, nsq_row, Sqrt, scale=float((NPG * EPS) ** 2)
                )
                # squared norms as [128, nblk] columns (for row scaling of C0)
                nsq_col = pss.tile([128, NBLK], f32, tag="small")
                for b in range(NBLK):
                    nc.tensor.matmul(
                        nsq_col[:, b : b + 1],
                        lhsT=x2T[:, ts(b, BLK)],
                        rhs=ones_col,
                        start=True,
                        stop=True,
                    )
                ncol = sb.tile([128, NBLK], f32, tag=f"ncol_{name}")
                nc.scalar.activation(ncol, nsq_col, Sqrt)
                invn = sb.tile([128, NBLK], f32, tag=f"invn_{name}")
                nc.vector.reciprocal(invn, ncol)
                return x_nat, xT, x2T, neps, invn

            xl_nat, xlT, xl2T, neps_l, invn_l = load_side(xl_d, "l")
            xr_nat, xrT, xr2T, neps_r, invn_r = load_side(xr_d, "r")

            # raw dot products S[i,j] per block, then C0 = relu(S)*mask
            S_ps = big.tile([128, NPC], f32, tag="big")
            for b in range(NBLK):
                nc.tensor.matmul(
                    S_ps[:, ts(b, BLK)],
                    lhsT=xlT[:, ts(b, BLK)],
                    rhs=xrT[:, ts(b, BLK)],
                    start=True,
                    stop=True,
                )
            C0r = sb.tile([128, NPC], f32, tag="C0r")
            nc.scalar.activation(C0r, S_ps, Relu)
            C0 = sb.tile([128, NPC], f32, tag="C0")
            nc.vector.tensor_mul(C0, C0r, mask)

            # right-agg operand: C0 * (1/|xl_i|) rows
            C0c = sb.tile([128, NPC], f32, tag="C0c")
            for b in range(NBLK):
                nc.vector.tensor_scalar_mul(
                    C0c[:, ts(b, BLK)], C0[:, ts(b, BLK)], invn_l[:, b : b + 1]
                )
            # left-agg operand: C0^T * (1/|xr_j|) rows
            C0T_ps = tp.tile([128, NPC], f32, tag="tp")
            for b in range(NBLK):
                nc.tensor.transpose(
                    out=C0T_ps[:, ts(b, BLK)], in_=C0[:, ts(b, BLK)], identity=ident
                )
            C0T = sb.tile([128, NPC], f32, tag="C0T")
            nc.scalar.copy(out=C0T, in_=C0T_ps)
            C0Tc = sb.tile([128, NPC], f32, tag="C0Tc")
            for b in range(NBLK):
                nc.vector.tensor_scalar_mul(
                    C0Tc[:, ts(b, BLK)], C0T[:, ts(b, BLK)], invn_r[:, b : b + 1]
                )

            # aggregations: glT[d,i] = sum_j xr[j,d]*C0Tc[j,i] (and row sums)
            glT_ps = big.tile([128, NPC], f32, tag="big")
            Drow_ps = pss.tile([1, NPC], f32, tag="small")
            for b in range(NBLK):
                nc.tensor.matmul(
                    glT_ps[:, ts(b, BLK)],
                    lhsT=xr_nat[:, b, :],
                    rhs=C0Tc[:, ts(b, BLK)],
                    start=True,
                    stop=True,
                )
                nc.tensor.matmul(
                    Drow_ps[:, ts(b, BLK)],
                    lhsT=ones_col,
                    rhs=C0Tc[:, ts(b, BLK)],
                    start=True,
                    stop=True,
                )
            def normalize_g(gT_ps, row_ps, neps, name):
                dadj = sb.tile([1, NPC], f32, tag=f"dadj_{name}")
                nc.vector.tensor_add(dadj, row_ps, neps)
                invd = sb.tile([1, NPC], f32, tag=f"invd_{name}")
                nc.vector.reciprocal_approx_fast(out=invd, in_=dadj)
                bc = sb.tile([128, NPC], f32, tag=f"bc_{name}")
                nc.gpsimd.partition_broadcast(bc, invd)
                gT = sb.tile([128, NPC], f32, tag=f"gT_{name}")
                nc.vector.tensor_mul(gT, gT_ps, bc)
                return gT

            glT = normalize_g(glT_ps, Drow_ps, neps_l, "l")

            grT_ps = big.tile([128, NPC], f32, tag="big")
            Erow_ps = pss.tile([1, NPC], f32, tag="small")
            for b in range(NBLK):
                nc.tensor.matmul(
                    grT_ps[:, ts(b, BLK)],
                    lhsT=xl_nat[:, b, :],
                    rhs=C0c[:, ts(b, BLK)],
                    start=True,
                    stop=True,
                )
                nc.tensor.matmul(
                    Erow_ps[:, ts(b, BLK)],
                    lhsT=ones_col,
                    rhs=C0c[:, ts(b, BLK)],
                    start=True,
                    stop=True,
                )
            grT = normalize_g(grT_ps, Erow_ps, neps_r, "r")

            def finish_side(xT, x2T, gT, out_dram, name):
                pT = sb.tile([128, NPC], f32, tag=f"pT_{name}")
                nc.vector.tensor_mul(pT, xT, gT)
                g2T = sb.tile([128, NPC], f32, tag=f"g2T_{name}")
                nc.gpsimd.tensor_mul(g2T, gT, gT)
                num_ps = ein.tile([128, NPC], f32, tag="ein")
                nc.tensor.matmul(num_ps, lhsT=w2t, rhs=pT, start=True, stop=True)
                dt_ps = ein.tile([128, NPC], f32, tag="ein")
                nc.tensor.matmul(dt_ps, lhsT=w2t, rhs=x2T, start=True, stop=True)
                dg_ps = ein.tile([128, NPC], f32, tag="ein")
                nc.tensor.matmul(dg_ps, lhsT=w2t, rhs=g2T, start=True, stop=True)
                dt = sb.tile([128, NPC], f32, tag=f"dts_{name}")
                nc.scalar.activation(dt, dt_ps, Sqrt, bias=eps_col[:])
                dg = sb.tile([128, NPC], f32, tag=f"dgs_{name}")
                nc.scalar.activation(dg, dg_ps, Sqrt, bias=eps_col[:])
                den = sb.tile([128, NPC], f32, tag=f"den_{name}")
                nc.vector.tensor_mul(den, dt, dg)
                inv = sb.tile([128, NPC], f32, tag=f"inv_{name}")
                nc.vector.reciprocal_approx_fast(out=inv, in_=den)
                oT = sb.tile([128, NPC], f32, tag=f"oT_{name}")
                nc.vector.tensor_mul(oT, num_ps, inv)
                # transpose back to [node, out] and store
                of_ps = tp.tile([128, NPC], f32, tag="tp")
                for b in range(NBLK):
                    nc.tensor.transpose(
                        out=of_ps[:, ts(b, BLK)],
                        in_=oT[:, ts(b, BLK)],
                        identity=ident,
                    )
                of = sb.tile([128, NPC], f32, tag=f"of_{name}")
                nc.scalar.copy(out=of, in_=of_ps)
                nc.sync.dma_start(
                    out=out_dram[:].rearrange("(b p) d -> p b d", p=BLK),
                    in_=of.rearrange("p (b d) -> p b d", b=NBLK),
                )

            finish_side(xlT, xl2T, glT, out1_d, "o1")
            finish_side(xrT, xr2T, grT, out2_d, "o2")

    nc.compile()
    return nc


def _edges_are_dense_bipartite(edge_row, edge_col):
    E = B * NPG * NPG
    if edge_row.shape != (E,) or edge_col.shape != (E,):
        return False
    b = np.arange(B, dtype=np.int64)[:, None, None]
    i = np.arange(NPG, dtype=np.int64)[None, :, None]
    j = np.arange(NPG, dtype=np.int64)[None, None, :]
    er = np.broadcast_to(b * NPG + i, (B, NPG, NPG)).reshape(-1)
    ec = np.broadcast_to(b * NPG + j, (B, NPG, NPG)).reshape(-1)
    return np.array_equal(edge_row.astype(np.int64), er) and np.array_equal(
        edge_col.astype(np.int64), ec
    )


def _numpy_fallback(x_left, x_right, edge_row, edge_col, weight):
    """General (slow, host) implementation for arbitrary edge lists."""

    def cross(x_src, x_dst, src_idx, dst_idx):
        M = x_dst.shape[0]
        xi = x_dst[dst_idx]
        xj = x_src[src_idx]
        nrm = np.maximum(
            np.linalg.norm(xi, axis=-1, keepdims=True)
            * np.linalg.norm(xj, axis=-1, keepdims=True),
            EPS,
        )
        coef = np.maximum((xi * xj).sum(-1, keepdims=True) / nrm, 0.0)
        coef_sum = np.zeros((M, 1), np.float32)
        np.add.at(coef_sum, dst_idx, coef + EPS)
        norm_coef = coef / coef_sum[dst_idx]
        gx = np.zeros_like(x_dst)
        np.add.at(gx, dst_idx, norm_coef * xj)
        w2 = weight * weight
        num = (x_dst * gx) @ w2.T
        den_t = np.sqrt((x_dst * x_dst) @ w2.T + EPS)
        den_g = np.sqrt((gx * gx) @ w2.T + EPS)
        return (num / np.maximum(den_t * den_g, EPS)).astype(np.float32)

    o1 = cross(x_right, x_left, edge_col, edge_row)
    o2 = cross(x_left, x_right, edge_row, edge_col)
    return o1, o2


def _make_mask4():
    m = np.zeros((BLK, BLK), np.float32)
    for g in range(BLK // NPG):
        m[g * NPG : (g + 1) * NPG, g * NPG : (g + 1) * NPG] = 1.0
    return np.tile(m, (1, NBLK)).copy()


def kernel(**inputs):
    x_left = np.ascontiguousarray(np.asarray(inputs["x_left"], np.float32))
    x_right = np.ascontiguousarray(np.asarray(inputs["x_right"], np.float32))
    edge_row = np.asarray(inputs["edge_row"])
    edge_col = np.asarray(inputs["edge_col"])
    weight = np.ascontiguousarray(np.asarray(inputs["weight"], np.float32))

    if not _edges_are_dense_bipartite(edge_row, edge_col):
        return _numpy_fallback(x_left, x_right, edge_row, edge_col, weight)

    from concourse.bass_utils import run_bass_kernel_spmd

    if "nc" not in _CACHE:
        _CACHE["nc"] = _build_bass()
    nc = _CACHE["nc"]

    w2t = np.ascontiguousarray((weight * weight).T.astype(np.float32))
    mask4 = _make_mask4()
    in_maps = []
    for k in range(NCORES):
        sl = slice(k * NPC, (k + 1) * NPC)
        in_maps.append(
            {
                "xl": np.ascontiguousarray(x_left[sl]),
                "xr": np.ascontiguousarray(x_right[sl]),
                "w2t": w2t,
                "mask4": mask4,
            }
        )
    res = run_bass_kernel_spmd(nc, in_maps, list(range(NCORES)))
    out1 = np.concatenate([res.results[k]["out1"] for k in range(NCORES)], axis=0)
    out2 = np.concatenate([res.results[k]["out2"] for k in range(NCORES)], axis=0)
    return out1, out2


# revision 10
# speedup vs baseline: 1.0011x; 1.0011x over previous
"""CrossGraphConvolution kernel for Trainium2 (Bass/Tile), 8-core SPMD.

Problem: B=128 graph pairs, NPG=32 nodes per side per graph, D=OUT=128.
Edges are dense block-bipartite within each graph pair (left i <-> right j).

Math per graph pair (both directions share the cosine matrix):
  C[i,j]   = relu(cos(xl_i, xr_j))            (32x32 per graph)
  nc[i,j]  = C/(rowsum(C)+32*eps)  -> gl_i = sum_j nc*xr_j
  nc'[i,j] = C/(colsum(C)+32*eps)  -> gr_j = sum_i nc'*xl_i
  out1[i,o] = (sum_d xl*gl*w2[o]) / sqrt((sum_d xl^2*w2[o])+e) /
              sqrt((sum_d gl^2*w2[o])+e),  w2 = weight**2; same for out2.

Sharding: data-parallel over graphs; core k handles graphs [16k, 16k+16),
processed as 4 "blocks" of 4 graphs = 128 nodes per side, batched into
[128, 512] tiles wherever the free dim allows.

Normalization trick: keep C completely unscaled (C0 = relu(S_raw)*mask);
fold 1/|x| scales into per-partition row scalings of C0 / C0^T, and the
row-sum denominators via an appended ones-reduction matmul; fold the final
1/D into g via a gpsimd partition-broadcast multiply, so every later
operand is the exact ("raw") quantity and no per-free-element broadcast
scaling is ever needed.
"""

import sys

import numpy as np

for _p in ("/opt/trn_rl_repo",):
    if _p not in sys.path:
        sys.path.insert(0, _p)

B = 128
NPG = 32
D = 128
OUT = 128
EPS = 1e-6
NCORES = 8
GPC = B // NCORES          # graphs per core = 16
NPC = GPC * NPG            # nodes per side per core = 512
BLK = 128                  # nodes per block (4 graphs)
NBLK = NPC // BLK          # blocks per core = 4

_CACHE = {}


def _build_bass():
    import concourse.bass as bass
    import concourse.bacc as bacc
    import concourse.tile as tile
    from concourse import mybir
    from concourse.bass import ts
    from concourse.masks import make_identity

    f32 = mybir.dt.float32
    f32r = mybir.dt.float32r
    Sqrt = mybir.ActivationFunctionType.Sqrt
    Relu = mybir.ActivationFunctionType.Relu

    nc = bacc.Bacc(None)
    xl_d = nc.dram_tensor("xl", [NPC, D], f32, kind="ExternalInput")
    xr_d = nc.dram_tensor("xr", [NPC, D], f32, kind="ExternalInput")
    w2t_d = nc.dram_tensor("w2t", [D, OUT], f32, kind="ExternalInput")
    mask_d = nc.dram_tensor("mask4", [BLK, NPC], f32, kind="ExternalInput")
    out1_d = nc.dram_tensor("out1", [NPC, OUT], f32, kind="ExternalOutput")
    out2_d = nc.dram_tensor("out2", [NPC, OUT], f32, kind="ExternalOutput")

    with tile.TileContext(nc) as tc:
        with (
            tc.tile_pool(name="const", bufs=1) as const,
            tc.tile_pool(name="sb", bufs=1) as sb,
            tc.tile_pool(name="tp", bufs=2, space="PSUM") as tp,
            tc.tile_pool(name="big", bufs=2, space="PSUM") as big,
            tc.tile_pool(name="ein", bufs=3, space="PSUM") as ein,
            tc.tile_pool(name="pss", bufs=1, space="PSUM") as pss,
        ):
            ident = const.tile([128, 128], f32, tag="ident")
            make_identity(nc, ident)
            w2t = const.tile([D, OUT], f32, tag="w2t")
            nc.sync.dma_start(out=w2t, in_=w2t_d[:])
            mask = const.tile([BLK, NPC], f32, tag="mask")
            nc.sync.dma_start(out=mask, in_=mask_d[:])
            ones_col = const.tile([128, 1], f32, tag="ones")
            nc.vector.memset(ones_col, 1.0)
            eps_col = const.tile([128, 1], f32, tag="eps")
            nc.vector.memset(eps_col, EPS)
            # pin the ACT table set containing Sqrt (Relu/Copy are fillers
            # in every set) so only one ACT_TABLE_LOAD happens
            tiny_sqrt = const.tile([1, 1], f32, tag="tinysqrt")
            nc.scalar.activation(tiny_sqrt, eps_col[0:1, :], Sqrt)
            # warm-up transposes: absorb the Pool (identity-gen) and the
            # w2t DMA-queue waits on PE so no later matmul needs >1 wait
            scrap_ps = tp.tile([128, 128], f32, tag="tp")
            nc.tensor.transpose(out=scrap_ps, in_=ident, identity=ident)
            scrap2_ps = tp.tile([128, 128], f32, tag="tp")
            nc.tensor.transpose(out=scrap2_ps, in_=w2t, identity=ident)

            def load_side(xdram, name):
                # natural layout [node-in-block, block, d], one DMA
                x_nat = sb.tile([BLK, NBLK, D], f32, tag=f"xnat_{name}")
                nc.sync.dma_start(
                    out=x_nat, in_=xdram[:].rearrange("(b p) d -> p b d", p=BLK)
                )
                # transposed layout [d, node] for all blocks
                xT_ps = tp.tile([128, NPC], f32, tag="tp")
                for b in range(NBLK):
                    nc.tensor.transpose(
                        out=xT_ps[:, ts(b, BLK)], in_=x_nat[:, b, :], identity=ident
                    )
                xT = sb.tile([128, NPC], f32, tag=f"xT_{name}")
                nc.scalar.copy(out=xT, in_=xT_ps)
                x2T = sb.tile([128, NPC], f32, tag=f"x2T_{name}")
                nc.vector.tensor_mul(x2T, xT, xT)
                # squared norms as a [1, 512] row (for the +32eps*|x| term)
                nsq_row = pss.tile([1, NPC], f32, tag="small")
                nc.tensor.matmul(
                    nsq_row, lhsT=ones_col, rhs=x2T, start=True, stop=True
                )
                neps = sb.tile([1, NPC], f32, tag=f"neps_{name}")
                nc.scalar.activation(
                    neps, nsq_row, Sqrt, scale=float((NPG * EPS) ** 2)
                )
                # squared norms as [128, nblk] columns (for row scaling of C0)
                nsq_col = pss.tile([128, NBLK], f32, tag="small")
                for b in range(NBLK):
                    nc.tensor.matmul(
                        nsq_col[:, b : b + 1],
                        lhsT=x2T[:, ts(b, BLK)],
                        rhs=ones_col,
                        start=True,
                        stop=True,
                    )
                ncol = sb.tile([128, NBLK], f32, tag=f"ncol_{name}")
                nc.scalar.activation(ncol, nsq_col, Sqrt)
                invn = sb.tile([128, NBLK], f32, tag=f"invn_{name}")
                nc.vector.reciprocal(invn, ncol)
                return x_nat, xT, x2T, neps, invn

            xl_nat, xlT, xl2T, neps_l, invn_l = load_side(xl_d, "l")
            xr_nat, xrT, xr2T, neps_r, invn_r = load_side(xr_d, "r")

            # raw dot products S[i,j] per block, then C0 = relu(S)*mask
            S_ps = big.tile([128, NPC], f32, tag="big")
            for b in range(NBLK):
                nc.tensor.matmul(
                    S_ps[:, ts(b, BLK)],
                    lhsT=xlT[:, ts(b, BLK)],
                    rhs=xrT[:, ts(b, BLK)],
                    start=True,
                    stop=True,
                )
            C0r = sb.tile([128, NPC], f32, tag="C0r")
            nc.scalar.activation(C0r, S_ps, Relu)
            C0 = sb.tile([128, NPC], f32, tag="C0")
            nc.gpsimd.tensor_mul(C0, C0r, mask)

            # right-agg operand: C0 * (1/|xl_i|) rows
            C0c = sb.tile([128, NPC], f32, tag="C0c")
            for b in range(NBLK):
                nc.vector.tensor_scalar_mul(
                    C0c[:, ts(b, BLK)], C0[:, ts(b, BLK)], invn_l[:, b : b + 1]
                )
            # left-agg operand: C0^T * (1/|xr_j|) rows
            C0T_ps = tp.tile([128, NPC], f32, tag="tp")
            for b in range(NBLK):
                nc.tensor.transpose(
                    out=C0T_ps[:, ts(b, BLK)], in_=C0[:, ts(b, BLK)], identity=ident
                )
            C0T = sb.tile([128, NPC], f32, tag="C0T")
            nc.scalar.copy(out=C0T, in_=C0T_ps)
            C0Tc = sb.tile([128, NPC], f32, tag="C0Tc")
            for b in range(NBLK):
                nc.vector.tensor_scalar_mul(
                    C0Tc[:, ts(b, BLK)], C0T[:, ts(b, BLK)], invn_r[:, b : b + 1]
                )

            # aggregations: glT[d,i] = sum_j xr[j,d]*C0Tc[j,i] (and row sums)
            glT_ps = big.tile([128, NPC], f32, tag="big")
            Drow_ps = pss.tile([1, NPC], f32, tag="small")
            for b in range(NBLK):
                nc.tensor.matmul(
                    glT_ps[:, ts(b, BLK)],
                    lhsT=xr_nat[:, b, :],
                    rhs=C0Tc[:, ts(b, BLK)],
                    start=True,
                    stop=True,
                )
                nc.tensor.matmul(
                    Drow_ps[:, ts(b, BLK)],
                    lhsT=ones_col,
                    rhs=C0Tc[:, ts(b, BLK)],
                    start=True,
                    stop=True,
                )
            def normalize_g(gT_ps, row_ps, neps, name):
                dadj = sb.tile([1, NPC], f32, tag=f"dadj_{name}")
                nc.vector.tensor_add(dadj, row_ps, neps)
                invd = sb.tile([1, NPC], f32, tag=f"invd_{name}")
                nc.vector.reciprocal_approx_fast(out=invd, in_=dadj)
                bc = sb.tile([128, NPC], f32, tag=f"bc_{name}")
                nc.gpsimd.partition_broadcast(bc, invd)
                gT = sb.tile([128, NPC], f32, tag=f"gT_{name}")
                nc.vector.tensor_mul(gT, gT_ps, bc)
                return gT

            glT = normalize_g(glT_ps, Drow_ps, neps_l, "l")

            grT_ps = big.tile([128, NPC], f32, tag="big")
            Erow_ps = pss.tile([1, NPC], f32, tag="small")
            for b in range(NBLK):
                nc.tensor.matmul(
                    grT_ps[:, ts(b, BLK)],
                    lhsT=xl_nat[:, b, :],
                    rhs=C0c[:, ts(b, BLK)],
                    start=True,
                    stop=True,
                )
                nc.tensor.matmul(
                    Erow_ps[:, ts(b, BLK)],
                    lhsT=ones_col,
                    rhs=C0c[:, ts(b, BLK)],
                    start=True,
                    stop=True,
                )
            grT = normalize_g(grT_ps, Erow_ps, neps_r, "r")

            def finish_side(xT, x2T, gT, out_dram, name):
                pT = sb.tile([128, NPC], f32, tag=f"pT_{name}")
                nc.vector.tensor_mul(pT, xT, gT)
                g2T = sb.tile([128, NPC], f32, tag=f"g2T_{name}")
                nc.gpsimd.tensor_mul(g2T, gT, gT)
                num_ps = ein.tile([128, NPC], f32, tag="ein")
                nc.tensor.matmul(num_ps, lhsT=w2t, rhs=pT, start=True, stop=True)
                dt_ps = ein.tile([128, NPC], f32, tag="ein")
                nc.tensor.matmul(dt_ps, lhsT=w2t, rhs=x2T, start=True, stop=True)
                dg_ps = ein.tile([128, NPC], f32, tag="ein")
                nc.tensor.matmul(dg_ps, lhsT=w2t, rhs=g2T, start=True, stop=True)
                dt = sb.tile([128, NPC], f32, tag=f"dts_{name}")
                nc.scalar.activation(dt, dt_ps, Sqrt, bias=eps_col[:])
                dg = sb.tile([128, NPC], f32, tag=f"dgs_{name}")
                nc.scalar.activation(dg, dg_ps, Sqrt, bias=eps_col[:])
                den = sb.tile([128, NPC], f32, tag=f"den_{name}")
                nc.vector.tensor_mul(den, dt, dg)
                inv = sb.tile([128, NPC], f32, tag=f"inv_{name}")
                nc.vector.reciprocal_approx_fast(out=inv, in_=den)
                oT = sb.tile([128, NPC], f32, tag=f"oT_{name}")
                nc.vector.tensor_mul(oT, num_ps, inv)
                # transpose back to [node, out] and store
                of_ps = tp.tile([128, NPC], f32, tag="tp")
                for b in range(NBLK):
                    nc.tensor.transpose(
                        out=of_ps[:, ts(b, BLK)],
                        in_=oT[:, ts(b, BLK)],
                        identity=ident,
                    )
                of = sb.tile([128, NPC], f32, tag=f"of_{name}")
                nc.scalar.copy(out=of, in_=of_ps)
                nc.sync.dma_start(
                    out=out_dram[:].rearrange("(b p) d -> p b d", p=BLK),
                    in_=of.rearrange("p (b d) -> p b d", b=NBLK),
                )

            finish_side(xlT, xl2T, glT, out1_d, "o1")
            finish_side(xrT, xr2T, grT, out2_d, "o2")

    nc.compile()
    return nc


def _edges_are_dense_bipartite(edge_row, edge_col):
    E = B * NPG * NPG
    if edge_row.shape != (E,) or edge_col.shape != (E,):
        return False
    b = np.arange(B, dtype=np.int64)[:, None, None]
    i = np.arange(NPG, dtype=np.int64)[None, :, None]
    j = np.arange(NPG, dtype=np.int64)[None, None, :]
    er = np.broadcast_to(b * NPG + i, (B, NPG, NPG)).reshape(-1)
    ec = np.broadcast_to(b * NPG + j, (B, NPG, NPG)).reshape(-1)
    return np.array_equal(edge_row.astype(np.int64), er) and np.array_equal(
        edge_col.astype(np.int64), ec
    )


def _numpy_fallback(x_left, x_right, edge_row, edge_col, weight):
    """General (slow, host) implementation for arbitrary edge lists."""

    def cross(x_src, x_dst, src_idx, dst_idx):
        M = x_dst.shape[0]
        xi = x_dst[dst_idx]
        xj = x_src[src_idx]
        nrm = np.maximum(
            np.linalg.norm(xi, axis=-1, keepdims=True)
            * np.linalg.norm(xj, axis=-1, keepdims=True),
            EPS,
        )
        coef = np.maximum((xi * xj).sum(-1, keepdims=True) / nrm, 0.0)
        coef_sum = np.zeros((M, 1), np.float32)
        np.add.at(coef_sum, dst_idx, coef + EPS)
        norm_coef = coef / coef_sum[dst_idx]
        gx = np.zeros_like(x_dst)
        np.add.at(gx, dst_idx, norm_coef * xj)
        w2 = weight * weight
        num = (x_dst * gx) @ w2.T
        den_t = np.sqrt((x_dst * x_dst) @ w2.T + EPS)
        den_g = np.sqrt((gx * gx) @ w2.T + EPS)
        return (num / np.maximum(den_t * den_g, EPS)).astype(np.float32)

    o1 = cross(x_right, x_left, edge_col, edge_row)
    o2 = cross(x_left, x_right, edge_row, edge_col)
    return o1, o2


def _make_mask4():
    m = np.zeros((BLK, BLK), np.float32)
    for g in range(BLK // NPG):
        m[g * NPG : (g + 1) * NPG, g * NPG : (g + 1) * NPG] = 1.0
    return np.tile(m, (1, NBLK)).copy()


def kernel(**inputs):
    x_left = np.ascontiguousarray(np.asarray(inputs["x_left"], np.float32))
    x_right = np.ascontiguousarray(np.asarray(inputs["x_right"], np.float32))
    edge_row = np.asarray(inputs["edge_row"])
    edge_col = np.asarray(inputs["edge_col"])
    weight = np.ascontiguousarray(np.asarray(inputs["weight"], np.float32))

    if not _edges_are_dense_bipartite(edge_row, edge_col):
        return _numpy_fallback(x_left, x_right, edge_row, edge_col, weight)

    from concourse.bass_utils import run_bass_kernel_spmd

    if "nc" not in _CACHE:
        _CACHE["nc"] = _build_bass()
    nc = _CACHE["nc"]

    w2t = np.ascontiguousarray((weight * weight).T.astype(np.float32))
    mask4 = _make_mask4()
    in_maps = []
    for k in range(NCORES):
        sl = slice(k * NPC, (k + 1) * NPC)
        in_maps.append(
            {
                "xl": np.ascontiguousarray(x_left[sl]),
                "xr": np.ascontiguousarray(x_right[sl]),
                "w2t": w2t,
                "mask4": mask4,
            }
        )
    res = run_bass_kernel_spmd(nc, in_maps, list(range(NCORES)))
    out1 = np.concatenate([res.results[k]["out1"] for k in range(NCORES)], axis=0)
    out2 = np.concatenate([res.results[k]["out2"] for k in range(NCORES)], axis=0)
    return out1, out2
